# revision 18
# baseline (speedup 1.0000x reference)
"""Trainium2 Bass kernel for nn_ActionConditionedFactorizedTransition.

out[b,s] = sum_k belief[b,k] * softmax_s(q_{b,k} . key_s / sqrt(H))

Sharding: the num_states axis S=50000 is split across 8 cores (6250 each).
Each core computes keys for its shard, scores for all 2048 (b,k) queries
against its shard, exp with a fused row-sum (accum_out), an 8KB AllReduce
to form the global softmax normalizer Z, then a PSUM-accumulated
contraction with belief/Z producing out[:, shard]. Host concatenates.
"""

import numpy as np
import ml_dtypes

import concourse.bass as bass
import concourse.mybir as mybir
from concourse import bacc, bass_utils
from concourse.bass import ds, ts
from concourse.tile import TileContext

BF16 = mybir.dt.bfloat16
F32 = mybir.dt.float32
NBF16 = ml_dtypes.bfloat16

S = 50000
E = 128
H = 128
B = 16
K = 128
NCORES = 8
SL = S // NCORES          # 6250 states per core
J = B * K                 # 2048 query rows
MM_N = 512                # matmul moving-operand cols (1 psum bank fp32)
NSL = (SL + MM_N - 1) // MM_N      # 13 contraction/key slices
SPAN = 1536               # exp span (3 psum banks); 2 bufs + out_ps 2 = 8 banks
NSPAN = (SL + SPAN - 1) // SPAN    # 5 spans per chunk


def _body(nc, tc, embT, qTe, wkq, brow, smalls, out, cc_in, cc_out):
    with (
        tc.tile_pool(name="persist", bufs=1) as pp,
        tc.tile_pool(name="stream", bufs=2) as sp,
        tc.tile_pool(name="stage", bufs=1) as stp,
        tc.tile_pool(name="psum", bufs=2, space="PSUM") as ps,
        tc.tile_pool(name="psum_o", bufs=2, space="PSUM") as pso,
    ):
        # ---- small constants (packed: one DMA each to keep sem fan-in low)
        wkq_sb = pp.tile([E, 2 * H], BF16, tag="wkq")
        nc.sync.dma_start(out=wkq_sb[:], in_=wkq[:])
        wk_sb = wkq_sb[:, :H]
        wq_sb = wkq_sb[:, H:]
        # brow [1, 2H + MM_N] bf16: bkT | bqT(scaled) | ones  (host-baked)
        br_sb = pp.tile([1, 2 * H + MM_N], BF16, tag="brow")
        nc.sync.dma_start(out=br_sb[:], in_=brow[:])
        ones_r = br_sb[:, 2 * H :]
        sm_sb = pp.tile([H, 2 + B], F32, tag="smalls")
        nc.sync.dma_start(out=sm_sb[:], in_=smalls[:])
        w_sb = sm_sb[:, 2 : 2 + B]

        # ---- queries qT[h, j] = (q_emb @ (Wq*scale).T).T + bq*scale  (bf16)
        # bias added as a rank-1 K=1 matmul accumulated into the same psum.
        qT = pp.tile([H, J], BF16, tag="qT")
        for i in range(J // MM_N):  # 4
            qe_t = sp.tile([E, MM_N], BF16, tag="emb")
            nc.sync.dma_start(out=qe_t[:], in_=qTe[:, ts(i, MM_N)])
            qps = ps.tile([H, SPAN], F32, tag="ps")
            nc.tensor.matmul(qps[:, :MM_N], wq_sb[:], qe_t[:], start=True, stop=False)
            nc.tensor.matmul(
                qps[:, :MM_N], br_sb[:, H : 2 * H], ones_r[:, :MM_N],
                start=False, stop=True,
            )
            nc.scalar.copy(qT[:, ts(i, MM_N)], qps[:, :MM_N])

        # ---- keys kT[h, s] = (emb_shard @ Wk.T).T + bk  (bf16)
        kT = pp.tile([H, SL], BF16, tag="kT")
        for t in range(NSL):
            n = min(MM_N, SL - t * MM_N)
            e_t = sp.tile([E, MM_N], BF16, tag="emb")
            nc.sync.dma_start(out=e_t[:, :n], in_=embT[:, ds(t * MM_N, n)])
            kps = ps.tile([H, SPAN], F32, tag="ps")
            nc.tensor.matmul(kps[:, :n], wk_sb[:], e_t[:, :n], start=True, stop=False)
            nc.tensor.matmul(
                kps[:, :n], br_sb[:, :H], ones_r[:, :n], start=False, stop=True
            )
            nc.scalar.copy(kT[:, ds(t * MM_N, n)], kps[:, :n])

        # ---- scores + exp per chunk (chunk c == batch b, 128 k-rows)
        E_all = pp.tile([K, B * SL], BF16, tag="E")
        zp = pp.tile([K, B * NSPAN], F32, tag="zp")
        for c in range(B):
            for g in range(NSPAN):
                w0 = g * SPAN
                wn = min(SPAN, SL - w0)
                sps = ps.tile([K, SPAN], F32, tag="ps")
                nsub = (wn + MM_N - 1) // MM_N
                for u in range(nsub):
                    o = u * MM_N
                    n = min(MM_N, wn - o)
                    nc.tensor.matmul(
                        sps[:, ds(o, n)],
                        qT[:, ts(c, K)],
                        kT[:, ds(w0 + o, n)],
                        start=True,
                        stop=True,
                    )
                nc.scalar.activation(
                    E_all[:, ds(c * SL + w0, wn)],
                    sps[:, :wn],
                    mybir.ActivationFunctionType.Exp,
                    accum_out=zp[:, ds(c * NSPAN + g, 1)],
                )

        # ---- local Z[k, b] = sum_g zp; AllReduce across cores
        z_loc = pp.tile([K, B], F32, tag="z")
        nc.vector.reduce_sum(
            z_loc[:],
            zp[:].rearrange("k (b g) -> k b g", g=NSPAN),
            axis=mybir.AxisListType.X,
        )
        nc.sync.dma_start(out=cc_in[:], in_=z_loc[:])
        nc.gpsimd.collective_compute(
            "AllReduce",
            mybir.AluOpType.add,
            replica_groups=[list(range(NCORES))],
            ins=[cc_in[:]],
            outs=[cc_out[:]],
        )
        zg = pp.tile([K, B], F32, tag="zg")
        nc.sync.dma_start(out=zg[:], in_=cc_out[:])

        # ---- v = belief / Z  -> block-diagonal Vsel [K, B*B] bf16
        zr = pp.tile([K, B], F32, tag="zr")
        nc.vector.reciprocal(zr[:], zg[:])
        vf = pp.tile([K, B], F32, tag="vf")
        nc.vector.tensor_mul(vf[:], w_sb[:], zr[:])
        vsel = pp.tile([K, B * B], BF16, tag="vsel")
        nc.vector.memset(vsel[:], 0.0)
        for c in range(B):
            nc.vector.tensor_copy(vsel[:, ds(c * B + c, 1)], vf[:, ds(c, 1)])

        # ---- contraction: out[b, s] = sum_c sum_k Vsel[k, c*B+b] * E_c[k, s]
        for t in range(NSL):
            n = min(MM_N, SL - t * MM_N)
            ops = pso.tile([B, MM_N], F32, tag="out_ps")
            for c in range(B):
                nc.tensor.matmul(
                    ops[:, :n],
                    vsel[:, ts(c, B)],
                    E_all[:, ds(c * SL + t * MM_N, n)],
                    start=(c == 0),
                    stop=(c == B - 1),
                )
            st = stp.tile([B, MM_N], F32, tag="st")
            nc.scalar.copy(st[:, :n], ops[:, :n])
            nc.sync.dma_start(out=out[:, ds(t * MM_N, n)], in_=st[:, :n])


def build_nc():
    nc = bacc.Bacc(
        "TRN2",
        target_bir_lowering=False,
        debug=False,
        num_devices=NCORES,
        dynamic_dma_scratch_size=2048,
    )
    embT = nc.dram_tensor("embT", [E, SL], BF16, kind="ExternalInput")
    qTe = nc.dram_tensor("qTe", [E, J], BF16, kind="ExternalInput")
    wkq = nc.dram_tensor("wkq", [E, 2 * H], BF16, kind="ExternalInput")
    brow = nc.dram_tensor("brow", [1, 2 * H + MM_N], BF16, kind="ExternalInput")
    smalls = nc.dram_tensor("smalls", [H, 2 + B], F32, kind="ExternalInput")
    out = nc.dram_tensor("out", [B, SL], F32, kind="ExternalOutput")
    cc_in = nc.dram_tensor("cc_in", [K, B], F32)
    cc_out = nc.dram_tensor("cc_out", [K, B], F32, addr_space="Shared")
    with TileContext(nc) as tc:
        _body(nc, tc, embT, qTe, wkq, brow, smalls, out, cc_in, cc_out)
    nc.compile()
    return nc


_NC = None


def _get_nc():
    global _NC
    if _NC is None:
        _NC = build_nc()
    return _NC


def make_in_maps(state_emb, Wk, bk, Wq, bq, state_belief, state_idcs):
    state_emb = np.asarray(state_emb, dtype=np.float32)
    Wk = np.asarray(Wk, dtype=np.float32)
    Wq = np.asarray(Wq, dtype=np.float32)
    bk_ = np.asarray(bk, dtype=np.float32).reshape(H, 1)
    bq_ = np.asarray(bq, dtype=np.float32).reshape(H, 1)
    belief = np.asarray(state_belief, dtype=np.float32)
    idcs = np.asarray(state_idcs).reshape(-1).astype(np.int64)

    scale = np.float32(1.0 / np.sqrt(H))
    q_emb = state_emb[idcs]                                   # [J, E]
    qTe = np.ascontiguousarray(q_emb.T).astype(NBF16)         # [E, J]
    wkq_ = np.concatenate([Wk.T, (Wq * scale).T], axis=1).astype(NBF16)  # [E, 2H]
    brow_ = np.concatenate(
        [bk_.reshape(1, H), (bq_ * scale).reshape(1, H), np.ones((1, MM_N), np.float32)],
        axis=1,
    ).astype(NBF16)                                            # [1, 2H + MM_N]
    smalls_ = np.concatenate([bk_, bq_ * scale, belief.T], axis=1).astype(np.float32)

    in_maps = []
    for m in range(NCORES):
        embT_m = np.ascontiguousarray(
            state_emb[m * SL : (m + 1) * SL].T
        ).astype(NBF16)                                        # [E, SL]
        in_maps.append(dict(embT=embT_m, qTe=qTe, wkq=wkq_, brow=brow_, smalls=smalls_))
    return in_maps


def kernel(state_emb, Wk, bk, Wq, bq, state_belief, state_idcs, action):
    in_maps = make_in_maps(state_emb, Wk, bk, Wq, bq, state_belief, state_idcs)
    nc = _get_nc()
    res = bass_utils.run_bass_kernel_spmd(nc, in_maps, core_ids=list(range(NCORES)))
    outs = [np.asarray(res.results[m]["out"]) for m in range(NCORES)]
    return np.concatenate(outs, axis=1).astype(np.float32)
